# revision 18
# baseline (speedup 1.0000x reference)
"""Trainium2 Bass kernel for ChannelTransformerBlock (restructured).

Block: x -> LN1 -> qkv -> channel-attention (48x48 per head, l2-normalized
q/k over tokens) -> proj -> +x -> LN2 -> fc1 -> gelu -> fc2 -> +x2.

Sharding: pure data-parallel over batch (16 batches -> 2 per core, 8 cores,
no collectives).

Key structural ideas (vs the v0 kernel):
  - Scores via the Gram matrix: attn_h = Wk_h^T (h^T h) Wq_h, so the
    token-major q/k tensors are never materialized. G = h^T h accumulates in
    PSUM across LN1's token tiles; the per-head 96x96 algebra V_h =
    [Wq|Wk]_h^T G [Wq|Wk]_h yields scores (S^T block) AND the l2-norm
    factors (diagonal) at once. G/T/V matmuls run float32r (full rate,
    better mantissa than bf16 for the big-diagonal Gram values).
  - S^T orientation feeds the (attn*rs)@Wv'^T matmul directly (lhsT = e1T),
    and a ones-column appended to Wv' makes the softmax denominator fall
    out of the same matmul -> no PE transposes in the softmax path.
  - attnout is computed token-major (ZT[n, r] = sum_c h^T[c,n] M^T[c,r]);
    the torch transpose(1,2).reshape scramble then reduces to stride-3
    column slices of ZT used directly as proj's stationary operand.
  - fc2 runs activation-stationary (lhsT = g4 tiles) producing token-major
    output on the grouped residual tiles -> no transposes after the MLP.
  - The remaining layout transposes (LN1/LN2 token-major -> channel-major,
    mp -> mftT) run on the DMA engines (XBAR dma_start_transpose), not PE.
  - Residual trunk x/x2 stays f32 in SBUF (precision), MLP processes
    8-group blocks so fc2's lhsT slices are full 128-wide.
"""

import sys

import numpy as np

try:
    import concourse.bass as bass  # noqa: F401
except ImportError:  # pragma: no cover
    for _p in ("/opt/trn_rl_repo", "/root/.axon_site/_ro/trn_rl_repo"):
        if _p not in sys.path:
            sys.path.insert(0, _p)

import ml_dtypes
from contextlib import ExitStack

import concourse.bass as bass
import concourse.mybir as mybir
from concourse import bacc
import concourse.tile as tile
from concourse.bass import ts
from concourse.bass_utils import run_bass_kernel_spmd
from concourse.masks import make_identity

F32 = mybir.dt.float32
F32R = mybir.dt.float32r
BF16 = mybir.dt.bfloat16
FP8 = mybir.dt.float8e4
AF = mybir.ActivationFunctionType
ALU = mybir.AluOpType
DR = mybir.MatmulPerfMode.DoubleRow

# Problem constants (hardcoded per task spec).
B, N, C = 16, 4096, 384
H, HD = 8, 48
HID = 4 * C
SCALE = HD ** -0.5
EPS = 1e-5
NCORES = 8
BL = B // NCORES          # batches per core
P = 128
NT = N // P               # token tiles per batch (32)
NTH = NT // 2             # token tiles per half (16)
CK = C // P               # channel chunks (3)
HK = HID // P             # hidden chunks (12)
GBLK = 8                  # groups per MLP block
NBLKS = NT // GBLK        # 4 MLP blocks
BTOK = P * GBLK           # tokens per MLP block (1024)

FP8_MLP = True            # fc2 in fp8 DoubleRow
GRAM_DT = mybir.dt.bfloat16   # bisect: float32r vs bfloat16 for the Gram path
XBAR_T = True                 # XBAR dma transposes (PE transposes if False)


def build_program(apply_pjb: bool, nbatch: int = BL):
    """Emit the per-core Bass/Tile program. Same NEFF runs on all 8 cores.

    Token tiles are "grouped": grouped tile g holds tokens {32*a + g,
    a=0..127} with a on partitions; channel-major free axes hold true token
    order n = 32a + g.
    """
    nc = bacc.Bacc()

    x_d = nc.declare_dram_parameter("x", [nbatch, N, C], F32, isOutput=False)
    wqk_d = nc.declare_dram_parameter("wqk", [C, 2 * C], GRAM_DT, isOutput=False)
    wvt_d = nc.declare_dram_parameter("wvt", [C, C + 1], BF16, isOutput=False)
    wpr_d = nc.declare_dram_parameter("wpr", [C, C], BF16, isOutput=False)
    pjb_d = nc.declare_dram_parameter("pjb", [C], F32, isOutput=False)
    wf1_d = nc.declare_dram_parameter("wf1", [C, HID], BF16, isOutput=False)
    f1b_d = nc.declare_dram_parameter("f1b", [HID], F32, isOutput=False)
    f2dt = FP8 if FP8_MLP else BF16
    wf2_d = nc.declare_dram_parameter("wf2", [HID, C], f2dt, isOutput=False)
    f2b_d = nc.declare_dram_parameter("f2b", [C], F32, isOutput=False)
    y_d = nc.declare_dram_parameter("y", [nbatch, N, C], F32, isOutput=True)

    with tile.TileContext(nc) as tc, ExitStack() as ctx:
        w = ctx.enter_context(tc.tile_pool(name="w", bufs=1))
        xres = ctx.enter_context(tc.tile_pool(name="xres", bufs=3))
        htp = ctx.enter_context(tc.tile_pool(name="htp", bufs=4))
        hTp = ctx.enter_context(tc.tile_pool(name="hTp", bufs=1))
        bigp = ctx.enter_context(tc.tile_pool(name="bigp", bufs=1))
        mftp = ctx.enter_context(tc.tile_pool(name="mftp", bufs=1))
        gsb = ctx.enter_context(tc.tile_pool(name="gsb", bufs=1))
        tsb = ctx.enter_context(tc.tile_pool(name="tsb", bufs=1))
        tiny = ctx.enter_context(tc.tile_pool(name="tiny", bufs=3))
        stgp = ctx.enter_context(tc.tile_pool(name="stgp", bufs=6))
        outp = ctx.enter_context(tc.tile_pool(name="outp", bufs=4))
        nbig = 5 if XBAR_T else 4
        ps_big = ctx.enter_context(tc.tile_pool(name="ps_big", bufs=nbig, space="PSUM"))
        ps_v = ctx.enter_context(tc.tile_pool(name="ps_v", bufs=2 if XBAR_T else 1, space="PSUM"))
        ps_m = ctx.enter_context(tc.tile_pool(name="ps_m", bufs=1, space="PSUM"))
        ps_t = None if XBAR_T else ctx.enter_context(
            tc.tile_pool(name="ps_t", bufs=2, space="PSUM"))
        dram = ctx.enter_context(tc.tile_pool(name="dram", bufs=2, space="DRAM"))

        # ---- constants / weights (loaded once) ----
        wqk_sb = w.tile([P, CK, 2 * C], GRAM_DT)
        nc.gpsimd.dma_start(wqk_sb[:], wqk_d.rearrange("(k p) m -> p k m", p=P))
        wvt_sb = w.tile([HD, H, C + 1], BF16)
        nc.gpsimd.dma_start(wvt_sb[:], wvt_d.rearrange("(t p) c -> p t c", p=HD))
        wpr_sb = w.tile([P, CK, C], BF16)
        nc.gpsimd.dma_start(wpr_sb[:], wpr_d.rearrange("(k p) m -> p k m", p=P))
        wf1_sb = w.tile([P, CK, HID], BF16)
        nc.gpsimd.dma_start(wf1_sb[:], wf1_d.rearrange("(k p) m -> p k m", p=P))
        f1b_sb = w.tile([P, HK], F32)
        nc.gpsimd.dma_start(f1b_sb[:], f1b_d.rearrange("(j p) -> p j", p=P))
        wf2_sb = w.tile([P, HK, C], f2dt)
        nc.gpsimd.dma_start(wf2_sb[:], wf2_d.rearrange("(k p) m -> p k m", p=P))
        # fc2 bias broadcast over partitions (per-free-element in token-major)
        f2b_sb = w.tile([P, C], F32)
        _f2b = f2b_d[:]
        nc.gpsimd.dma_start(
            f2b_sb[:], bass.AP(tensor=_f2b.tensor, offset=_f2b.offset,
                               ap=[[0, P], [1, C]]))
        pjb_sb = None
        if apply_pjb:
            pjb_sb = w.tile([P, C], F32)
            _pjb = pjb_d[:]
            nc.gpsimd.dma_start(
                pjb_sb[:], bass.AP(tensor=_pjb.tensor, offset=_pjb.offset,
                                   ap=[[0, P], [1, C]]))

        ident = w.tile([P, P], BF16)
        make_identity(nc, ident[:])
        eps_sb = w.tile([P, 1], F32)
        nc.vector.memset(eps_sb[:], EPS)
        skb = w.tile([HD, H, HD], F32)

        def layernorm(src_ap, dst_ap):
            st = tiny.tile([P, 6], F32, tag="bnstats", name="st")
            nc.vector.bn_stats(st[:], src_ap)
            mv = tiny.tile([P, 2], F32, tag="bnaggr", name="mv")
            nc.vector.bn_aggr(mv[:], st[:])
            rs = tiny.tile([P, 1], F32, tag="rstd", name="rs")
            nc.scalar.activation(rs[:], mv[:, 1:2], AF.Sqrt,
                                 bias=eps_sb[:, 0:1], scale=1.0)
            nc.vector.reciprocal(rs[:], rs[:])
            nc.vector.tensor_scalar(dst_ap, src_ap,
                                    scalar1=mv[:, 0:1], scalar2=rs[:],
                                    op0=ALU.subtract, op1=ALU.mult)

        def ln_transpose(ht, hT, g, blocked):
            # token-major LN tile -> channel-major: XBAR transpose into
            # contiguous staging, then strided copy on GpSimd (XBAR output
            # APs must be last-dim contiguous). h1T free axis is true token
            # order (n = 32a + g); h2T uses MLP-block order
            # (g//8)*1024 + a*8 + g%8 so fc1's moving APs are contiguous.
            for j in range(CK):
                if blocked:
                    dst = hT[:, j, :].rearrange(
                        "p (b a e) -> p b e a", b=NBLKS, e=GBLK)[
                        :, g // GBLK, g % GBLK, :]
                else:
                    dst = hT[:, j, :].rearrange(
                        "p (a s) -> p s a", s=NT)[:, g, :]
                if XBAR_T:
                    stg = stgp.tile([P, P], BF16, tag="stg", name="stg")
                    nc.sync.dma_start_transpose(stg[:], ht[:, ts(j, P)])
                    nc.gpsimd.tensor_copy(dst, stg[:])
                else:
                    pt = ps_t.tile([P, P], BF16, tag="pt", name="pt")
                    nc.tensor.transpose(pt[:], ht[:, ts(j, P)], ident[:])
                    if (g + j) % 2 == 0:
                        nc.scalar.copy(dst, pt[:])
                    else:
                        nc.vector.tensor_copy(dst, pt[:])

        for b in range(nbatch):
            xg = x_d[b].rearrange("(a s) c -> a s c", s=NT)
            yg = y_d[b].rearrange("(a s) c -> a s c", s=NT)

            # ---- load x (grouped token tiles) in halves ----
            xh = []
            for hf in range(2):
                xt = xres.tile([P, NTH, C], F32, tag="x", name=f"xh{hf}")
                nc.gpsimd.dma_start(xt[:], xg[:, ts(hf, NTH), :])
                xh.append(xt)

            def x_ap(g):
                return xh[g // NTH][:, g % NTH, :]

            # ---- phase A: LN1 + h1T (DMA transpose) + G accumulation ----
            h1T = hTp.tile([P, CK, N], BF16, tag="hT", name="h1T")
            psG = [ps_big.tile([P, 512], F32, tag="big", name=f"psG{j}")
                   for j in range(CK)]
            for g in range(NT):
                ht = htp.tile([P, C], BF16, tag="ht", name="ht")
                layernorm(x_ap(g), ht[:])
                ln_transpose(ht, h1T, g, False)
                for j in range(CK):
                    nc.tensor.matmul(psG[j][:, :C], ht[:, ts(j, P)], ht[:, :C],
                                     start=(g == 0), stop=(g == NT - 1))

            # ---- G -> SBUF (f32), T = G @ [Wq|Wk] in f32r ----
            Gsb = gsb.tile([P, CK, C], GRAM_DT, tag="G", name="Gsb")
            for j in range(CK):
                if j % 2 == 0:
                    nc.vector.tensor_copy(Gsb[:, j, :], psG[j][:, :C])
                else:
                    nc.scalar.copy(Gsb[:, j, :], psG[j][:, :C])
            Tsb = tsb.tile([P, CK, 2 * C], GRAM_DT, tag="T", name="Tsb")
            for m in range(CK):
                for hf in range(2):
                    pT = ps_big.tile([P, 512], F32, tag="big", name="pT")
                    for jj in range(CK):
                        nc.tensor.matmul(
                            pT[:, :C], Gsb[:, jj, ts(m, P)],
                            wqk_sb[:, jj, ts(hf, C)],
                            start=(jj == 0), stop=(jj == CK - 1))
                    dst = Tsb[:, m, ts(hf, C)]
                    if (m * 2 + hf) % 2 == 0:
                        nc.vector.tensor_copy(dst, pT[:, :C])
                    else:
                        nc.scalar.copy(dst, pT[:, :C])

            # ---- per-head V_h = U_h^T T_h: scores (S^T) + norms (diag) ----
            norms2 = tiny.tile([2 * HD, H], F32, tag="norms", name="norms2")
            sTall = tiny.tile([HD, H, HD], F32, tag="sTall", name="sTall")
            for h in range(H):
                psV = ps_v.tile([2 * HD, 2 * HD], F32, tag="V", name="psV")
                for jj in range(CK):
                    nc.tensor.matmul(psV[:], wqk_sb[:, jj, ts(h, 2 * HD)],
                                     Tsb[:, jj, ts(h, 2 * HD)],
                                     start=(jj == 0), stop=(jj == CK - 1))
                nc.scalar.copy(sTall[:, h, :], psV[0:HD, HD:2 * HD])
                junk = tiny.tile([2 * HD, 2 * HD], F32, tag="junk", name="junk")
                nc.vector.tensor_tensor(junk[:], psV[:],
                                        ident[0:2 * HD, 0:2 * HD], ALU.mult)
                junk2 = tiny.tile([2 * HD, 2 * HD], F32, tag="junk2",
                                  name="junk2")
                nc.scalar.activation(junk2[:], junk[:], AF.Identity,
                                     accum_out=norms2[:, h:h + 1])

            # ---- l2norm factors: rs = rsqrt(max(n2, eps)), k-side * SCALE ----
            rsall = tiny.tile([2 * HD, H], F32, tag="rsall", name="rsall")
            nc.vector.tensor_scalar_max(rsall[:], norms2[:], 1e-24)
            nc.scalar.activation(rsall[:], rsall[:], AF.Sqrt)
            nc.vector.reciprocal(rsall[:], rsall[:])
            # SCALE folded into the q-side factors (partition base 0; a
            # base-48 compute slice would be rejected by the verifier)
            nc.vector.tensor_scalar_mul(rsall[0:HD, :], rsall[0:HD, :], SCALE)
            # broadcast k-side factors along partitions via DRAM roundtrip
            # DRAM layout h-major (h*48 + d) so the broadcast reload is a
            # plain 2D stride-0-partition AP.
            s_dram = dram.tile([HD * H], F32, tag="s_dram", name="s_dram")
            _sd = s_dram[:]
            nc.gpsimd.dma_start(
                bass.AP(tensor=_sd.tensor, offset=_sd.offset,
                        ap=[[1, HD], [HD, H]]), rsall[HD:2 * HD, :])
            nc.gpsimd.dma_start(
                skb[:], bass.AP(tensor=_sd.tensor, offset=_sd.offset,
                                ap=[[0, HD], [1, H * HD]]))

            # ---- softmax + M^T columns (mftT), transposed via XBAR ----
            mftT = mftp.tile([P, CK, C], BF16, tag="mftT", name="mftT")
            for h in range(H):
                t1 = tiny.tile([HD, HD], F32, tag="t1", name="t1")
                nc.vector.scalar_tensor_tensor(
                    t1[:], sTall[:, h, :], rsall[0:HD, h:h + 1], skb[:, h, :],
                    op0=ALU.mult, op1=ALU.mult)
                e1T = tiny.tile([HD, HD], BF16, tag="e1T", name="e1T")
                nc.scalar.activation(e1T[:], t1[:], AF.Exp)
                psm = ps_m.tile([HD, C + 1], F32, tag="pm", name="psm")
                nc.tensor.matmul(psm[:], e1T[:], wvt_sb[:, h, :],
                                 start=True, stop=True)
                rsd = tiny.tile([HD, 1], F32, tag="rsd", name="rsd")
                nc.vector.reciprocal(rsd[:], psm[:, C:C + 1])
                mp = tiny.tile([HD, C], BF16, tag="mp", name="mp")
                nc.vector.tensor_scalar(mp[:], psm[:, 0:C],
                                        scalar1=rsd[:], scalar2=None,
                                        op0=ALU.mult)
                for kc in range(CK):
                    dst = mftT[:, kc, :].rearrange(
                        "p (d e) -> p e d", e=H)[:, h, :]
                    if XBAR_T:
                        stg = stgp.tile([P, P], BF16, tag="stg", name="stgm")
                        nc.sync.dma_start_transpose(stg[:, 0:HD],
                                                    mp[:, ts(kc, P)])
                        nc.gpsimd.tensor_copy(dst, stg[:, 0:HD])
                    else:
                        pt = ps_t.tile([P, P], BF16, tag="pt", name="ptm")
                        nc.tensor.transpose(pt[:, 0:HD], mp[:, ts(kc, P)],
                                            ident[0:HD, 0:HD])
                        if (h + kc) % 2 == 0:
                            nc.scalar.copy(dst, pt[:, 0:HD])
                        else:
                            nc.vector.tensor_copy(dst, pt[:, 0:HD])

            # ---- ZT[n, r] = sum_c h^T[c, n] M^T[c, r] (token-major) ----
            ZT = bigp.tile([P, NT, C], BF16, tag="big", name="ZT")
            for mu in range(NT):
                pz = ps_big.tile([P, 512], F32, tag="big", name="pz")
                for kc in range(CK):
                    nc.tensor.matmul(pz[:, :C], h1T[:, kc, ts(mu, P)],
                                     mftT[:, kc, :],
                                     start=(kc == 0), stop=(kc == CK - 1))
                if mu % 2 == 0:
                    nc.vector.tensor_copy(ZT[:, mu, :], pz[:, :C])
                else:
                    nc.scalar.copy(ZT[:, mu, :], pz[:, :C])

            # ---- proj via stride-3 ZT slices + residual + LN2 + h2T ----
            h2T = hTp.tile([P, CK, N], BF16, tag="hT", name="h2T")
            for g in range(NT):
                pp = ps_big.tile([P, 512], F32, tag="big", name="pp")
                for kj in range(CK):
                    q3 = 3 * g + kj
                    rho, mu = q3 // NT, q3 % NT
                    lhsT = ZT[:, mu, :].rearrange(
                        "p (a t) -> p t a", t=CK)[:, rho, :]
                    nc.tensor.matmul(pp[:, :C], lhsT, wpr_sb[:, kj, :],
                                     start=(kj == 0), stop=(kj == CK - 1))
                nc.vector.tensor_add(x_ap(g), pp[:, :C], x_ap(g))
                if apply_pjb:
                    nc.vector.tensor_add(x_ap(g), x_ap(g), pjb_sb[:])
                ht2 = htp.tile([P, C], BF16, tag="ht", name="ht2")
                layernorm(x_ap(g), ht2[:])
                ln_transpose(ht2, h2T, g, True)

            # ---- MLP in 8-group blocks: fc1 -> gelu -> fc2 (token-major) --
            for blk in range(NBLKS):
                g0 = blk * GBLK
                g4 = bigp.tile([P, HK, BTOK], f2dt, tag="big", name="g4")
                for m in range(HK):
                    pf = [ps_big.tile([P, 512], F32, tag="big", name=f"pf{i}")
                          for i in range(2)]
                    for kj in range(CK):
                        for i in range(2):
                            rhs = h2T[:, kj, blk * BTOK + i * 512:
                                      blk * BTOK + (i + 1) * 512]
                            nc.tensor.matmul(pf[i][:], wf1_sb[:, kj, ts(m, P)],
                                             rhs, start=(kj == 0),
                                             stop=(kj == CK - 1))
                    for i in range(2):
                        nc.scalar.activation(g4[:, m, ts(i, 512)], pf[i][:],
                                             AF.Gelu, bias=f1b_sb[:, m:m + 1],
                                             scale=1.0)
                g4s = g4.rearrange("p k (a e) -> p k e a", e=GBLK)
                for gg in range(GBLK):
                    g = g0 + gg
                    pf2 = ps_big.tile([P, 512], F32, tag="big", name="pf2")
                    if FP8_MLP:
                        g4e = g4.rearrange("p k (a e) -> p k e a", e=GBLK)
                        for u in range(HK // 2):
                            nc.tensor.matmul(
                                pf2[:, :C], g4e[:, 2 * u:2 * u + 2, gg, :],
                                wf2_sb[:, 2 * u:2 * u + 2, :],
                                start=(u == 0), stop=(u == HK // 2 - 1),
                                perf_mode=DR)
                    else:
                        for kj in range(HK):
                            nc.tensor.matmul(pf2[:, :C], g4s[:, kj, gg, :],
                                             wf2_sb[:, kj, :],
                                             start=(kj == 0),
                                             stop=(kj == HK - 1))
                    yt = outp.tile([P, C], F32, tag="yt", name="yt")
                    if FP8_MLP:
                        # descale the x64 fp8 weight prescale, add bias
                        nc.vector.scalar_tensor_tensor(
                            yt[:], pf2[:, :C], 1.0 / 64.0, f2b_sb[:],
                            op0=ALU.mult, op1=ALU.add)
                    else:
                        nc.vector.tensor_tensor(yt[:], pf2[:, :C], f2b_sb[:],
                                                ALU.add)
                    nc.vector.tensor_add(yt[:], yt[:], x_ap(g))
                    nc.sync.dma_start(yg[:, g, :], yt[:])

    nc.compile()
    return nc


def kernel_gram_np_dtype():
    return ml_dtypes.bfloat16 if GRAM_DT == mybir.dt.bfloat16 else None


def _prep_inputs(x, qkv_w, qkv_b, proj_w, proj_b, n1_g, n1_b, n2_g, n2_b,
                 fc1_w, fc1_b, fc2_w, fc2_b):
    """Host-side folding of LN affine params into the adjacent matmuls."""
    bf = ml_dtypes.bfloat16
    x = np.ascontiguousarray(np.asarray(x, np.float32))
    qkv_w = np.asarray(qkv_w, np.float32)
    qkv_b = np.asarray(qkv_b, np.float32)
    n1_g = np.asarray(n1_g, np.float32)
    n1_b = np.asarray(n1_b, np.float32)
    fc1_w = np.asarray(fc1_w, np.float32)
    wqk = n1_g[:, None] * qkv_w[:, :2 * C]
    # permute columns to per-head [q48 | k48] blocks (h*96 + {d, 48+d}) so
    # the V_h matmuls read contiguous slices
    perm = np.concatenate(
        [np.concatenate([h * HD + np.arange(HD), C + h * HD + np.arange(HD)])
         for h in range(H)])
    wqk = np.ascontiguousarray(wqk[:, perm])
    if kernel_gram_np_dtype() is not None:
        wqk = wqk.astype(kernel_gram_np_dtype())
    qkb = qkv_b[:2 * C] + n1_b @ qkv_w[:, :2 * C]
    if np.any(qkb != 0):
        raise NotImplementedError("nonzero q/k-bias not supported")
    wv = n1_g[:, None] * qkv_w[:, 2 * C:]
    vb = qkv_b[2 * C:] + n1_b @ qkv_w[:, 2 * C:]
    if np.any(vb != 0):
        raise NotImplementedError("nonzero v-bias not supported")
    wvt = np.concatenate(
        [np.ascontiguousarray(wv.T), np.ones((C, 1), np.float32)],
        axis=1).astype(bf)
    wf1 = (np.asarray(n2_g, np.float32)[:, None] * fc1_w).astype(bf)
    f1b = np.asarray(fc1_b, np.float32) + np.asarray(n2_b, np.float32) @ fc1_w
    pjb = np.asarray(proj_b, np.float32)
    apply_pjb = bool(np.any(pjb != 0))
    wf2 = np.asarray(fc2_w, np.float32)
    if FP8_MLP:
        wf2 = (wf2 * 64.0).astype(ml_dtypes.float8_e4m3)
    else:
        wf2 = wf2.astype(bf)
    common = {
        "wqk": wqk, "wvt": wvt,
        "wpr": np.asarray(proj_w, np.float32).astype(bf), "pjb": pjb,
        "wf1": wf1, "f1b": f1b.astype(np.float32),
        "wf2": wf2, "f2b": np.asarray(fc2_b, np.float32),
    }
    in_maps = []
    for c in range(NCORES):
        m = dict(common)
        m["x"] = x[c * BL:(c + 1) * BL]
        in_maps.append(m)
    return in_maps, apply_pjb


_CACHE = {}


def run(inputs: dict, trace: bool = False):
    in_maps, apply_pjb = _prep_inputs(**inputs)
    key = (apply_pjb,)
    if key not in _CACHE:
        _CACHE[key] = build_program(apply_pjb)
    nc = _CACHE[key]
    res = run_bass_kernel_spmd(nc, in_maps, core_ids=list(range(NCORES)),
                               trace=trace)
    y = np.concatenate([res.results[c]["y"] for c in range(NCORES)], axis=0)
    return y.astype(np.float32), res


def kernel(**inputs) -> np.ndarray:
    y, _ = run(inputs, trace=False)
    return y


# revision 19
# speedup vs baseline: 1.3606x; 1.3606x over previous
"""Trainium2 Bass kernel for ChannelTransformerBlock (restructured).

Block: x -> LN1 -> qkv -> channel-attention (48x48 per head, l2-normalized
q/k over tokens) -> proj -> +x -> LN2 -> fc1 -> gelu -> fc2 -> +x2.

Sharding: pure data-parallel over batch (16 batches -> 2 per core, 8 cores,
no collectives).

Key structural ideas (vs the v0 kernel):
  - Scores via the Gram matrix: attn_h = Wk_h^T (h^T h) Wq_h, so the
    token-major q/k tensors are never materialized. G = h^T h accumulates in
    PSUM across LN1's token tiles; the per-head 96x96 algebra V_h =
    [Wq|Wk]_h^T G [Wq|Wk]_h yields scores (S^T block) AND the l2-norm
    factors (diagonal) at once. G/T/V matmuls run float32r (full rate,
    better mantissa than bf16 for the big-diagonal Gram values).
  - S^T orientation feeds the (attn*rs)@Wv'^T matmul directly (lhsT = e1T),
    and a ones-column appended to Wv' makes the softmax denominator fall
    out of the same matmul -> no PE transposes in the softmax path.
  - attnout is computed token-major (ZT[n, r] = sum_c h^T[c,n] M^T[c,r]);
    the torch transpose(1,2).reshape scramble then reduces to stride-3
    column slices of ZT used directly as proj's stationary operand.
  - fc2 runs activation-stationary (lhsT = g4 tiles) producing token-major
    output on the grouped residual tiles -> no transposes after the MLP.
  - The remaining layout transposes (LN1/LN2 token-major -> channel-major,
    mp -> mftT) run on the DMA engines (XBAR dma_start_transpose), not PE.
  - Residual trunk x/x2 stays f32 in SBUF (precision), MLP processes
    8-group blocks so fc2's lhsT slices are full 128-wide.
"""

import sys

import numpy as np

try:
    import concourse.bass as bass  # noqa: F401
except ImportError:  # pragma: no cover
    for _p in ("/opt/trn_rl_repo", "/root/.axon_site/_ro/trn_rl_repo"):
        if _p not in sys.path:
            sys.path.insert(0, _p)

import ml_dtypes
from contextlib import ExitStack

import concourse.bass as bass
import concourse.mybir as mybir
from concourse import bacc
import concourse.tile as tile
from concourse.bass import ts
from concourse.bass_utils import run_bass_kernel_spmd
from concourse.masks import make_identity

F32 = mybir.dt.float32
F32R = mybir.dt.float32r
BF16 = mybir.dt.bfloat16
FP8 = mybir.dt.float8e4
AF = mybir.ActivationFunctionType
ALU = mybir.AluOpType
DR = mybir.MatmulPerfMode.DoubleRow

# Problem constants (hardcoded per task spec).
B, N, C = 16, 4096, 384
H, HD = 8, 48
HID = 4 * C
SCALE = HD ** -0.5
EPS = 1e-5
NCORES = 8
BL = B // NCORES          # batches per core
P = 128
NT = N // P               # token tiles per batch (32)
NTH = NT // 2             # token tiles per half (16)
CK = C // P               # channel chunks (3)
HK = HID // P             # hidden chunks (12)
GBLK = 8                  # groups per MLP block
NBLKS = NT // GBLK        # 4 MLP blocks
BTOK = P * GBLK           # tokens per MLP block (1024)

FP8_MLP = True            # fc2 in fp8 DoubleRow
GRAM_DT = mybir.dt.bfloat16   # bisect: float32r vs bfloat16 for the Gram path
XBAR_T = False                # XBAR transposes measured 1.2us serialized on SP; PE wins


def build_program(apply_pjb: bool, nbatch: int = BL):
    """Emit the per-core Bass/Tile program. Same NEFF runs on all 8 cores.

    Token tiles are "grouped": grouped tile g holds tokens {32*a + g,
    a=0..127} with a on partitions; channel-major free axes hold true token
    order n = 32a + g.
    """
    nc = bacc.Bacc()

    x_d = nc.declare_dram_parameter("x", [nbatch, N, C], F32, isOutput=False)
    wqk_d = nc.declare_dram_parameter("wqk", [C, 2 * C], GRAM_DT, isOutput=False)
    wvt_d = nc.declare_dram_parameter("wvt", [C, C + 1], BF16, isOutput=False)
    wpr_d = nc.declare_dram_parameter("wpr", [C, C], BF16, isOutput=False)
    pjb_d = nc.declare_dram_parameter("pjb", [C], F32, isOutput=False)
    wf1_d = nc.declare_dram_parameter("wf1", [C, HID], BF16, isOutput=False)
    f1b_d = nc.declare_dram_parameter("f1b", [HID], F32, isOutput=False)
    f2dt = FP8 if FP8_MLP else BF16
    wf2_d = nc.declare_dram_parameter("wf2", [HID, C], f2dt, isOutput=False)
    f2b_d = nc.declare_dram_parameter("f2b", [C], F32, isOutput=False)
    y_d = nc.declare_dram_parameter("y", [nbatch, N, C], F32, isOutput=True)

    with tile.TileContext(nc) as tc, ExitStack() as ctx:
        w = ctx.enter_context(tc.tile_pool(name="w", bufs=1))
        xres = ctx.enter_context(tc.tile_pool(name="xres", bufs=3))
        htp = ctx.enter_context(tc.tile_pool(name="htp", bufs=4))
        hTp = ctx.enter_context(tc.tile_pool(name="hTp", bufs=1))
        bigp = ctx.enter_context(tc.tile_pool(name="bigp", bufs=1))
        mftp = ctx.enter_context(tc.tile_pool(name="mftp", bufs=1))
        gsb = ctx.enter_context(tc.tile_pool(name="gsb", bufs=1))
        tsb = ctx.enter_context(tc.tile_pool(name="tsb", bufs=1))
        tiny = ctx.enter_context(tc.tile_pool(name="tiny", bufs=3))
        stgp = ctx.enter_context(tc.tile_pool(name="stgp", bufs=6))
        outp = ctx.enter_context(tc.tile_pool(name="outp", bufs=4))
        nbig = 5 if XBAR_T else 4
        ps_big = ctx.enter_context(tc.tile_pool(name="ps_big", bufs=nbig, space="PSUM"))
        ps_v = ctx.enter_context(tc.tile_pool(name="ps_v", bufs=2 if XBAR_T else 1, space="PSUM"))
        ps_m = ctx.enter_context(tc.tile_pool(name="ps_m", bufs=1, space="PSUM"))
        ps_t = None if XBAR_T else ctx.enter_context(
            tc.tile_pool(name="ps_t", bufs=2, space="PSUM"))
        dram = ctx.enter_context(tc.tile_pool(name="dram", bufs=2, space="DRAM"))

        # ---- constants / weights (loaded once) ----
        wqk_sb = w.tile([P, CK, 2 * C], GRAM_DT)
        nc.gpsimd.dma_start(wqk_sb[:], wqk_d.rearrange("(k p) m -> p k m", p=P))
        wvt_sb = w.tile([HD, H, C + 1], BF16)
        nc.gpsimd.dma_start(wvt_sb[:], wvt_d.rearrange("(t p) c -> p t c", p=HD))
        wpr_sb = w.tile([P, CK, C], BF16)
        nc.gpsimd.dma_start(wpr_sb[:], wpr_d.rearrange("(k p) m -> p k m", p=P))
        wf1_sb = w.tile([P, CK, HID], BF16)
        nc.gpsimd.dma_start(wf1_sb[:], wf1_d.rearrange("(k p) m -> p k m", p=P))
        f1b_sb = w.tile([P, HK], F32)
        nc.gpsimd.dma_start(f1b_sb[:], f1b_d.rearrange("(j p) -> p j", p=P))
        wf2_sb = w.tile([P, HK, C], f2dt)
        nc.gpsimd.dma_start(wf2_sb[:], wf2_d.rearrange("(k p) m -> p k m", p=P))
        # fc2 bias broadcast over partitions (per-free-element in token-major)
        f2b_sb = w.tile([P, C], F32)
        _f2b = f2b_d[:]
        nc.gpsimd.dma_start(
            f2b_sb[:], bass.AP(tensor=_f2b.tensor, offset=_f2b.offset,
                               ap=[[0, P], [1, C]]))
        pjb_sb = None
        if apply_pjb:
            pjb_sb = w.tile([P, C], F32)
            _pjb = pjb_d[:]
            nc.gpsimd.dma_start(
                pjb_sb[:], bass.AP(tensor=_pjb.tensor, offset=_pjb.offset,
                                   ap=[[0, P], [1, C]]))

        ident = w.tile([P, P], BF16)
        make_identity(nc, ident[:])
        eps_sb = w.tile([P, 1], F32)
        nc.vector.memset(eps_sb[:], EPS)
        skb = w.tile([HD, H, HD], F32)

        def layernorm(src_ap, dst_ap):
            st = tiny.tile([P, 6], F32, tag="bnstats", name="st")
            nc.vector.bn_stats(st[:], src_ap)
            mv = tiny.tile([P, 2], F32, tag="bnaggr", name="mv")
            nc.vector.bn_aggr(mv[:], st[:])
            rs = tiny.tile([P, 1], F32, tag="rstd", name="rs")
            nc.scalar.activation(rs[:], mv[:, 1:2], AF.Sqrt,
                                 bias=eps_sb[:, 0:1], scale=1.0)
            nc.vector.reciprocal(rs[:], rs[:])
            nc.vector.tensor_scalar(dst_ap, src_ap,
                                    scalar1=mv[:, 0:1], scalar2=rs[:],
                                    op0=ALU.subtract, op1=ALU.mult)

        def ln_transpose(ht, hT, g, blocked):
            # token-major LN tile -> channel-major: XBAR transpose into
            # contiguous staging, then strided copy on GpSimd (XBAR output
            # APs must be last-dim contiguous). h1T free axis is true token
            # order (n = 32a + g); h2T uses MLP-block order
            # (g//8)*1024 + a*8 + g%8 so fc1's moving APs are contiguous.
            for j in range(CK):
                if blocked:
                    dst = hT[:, j, :].rearrange(
                        "p (b a e) -> p b e a", b=NBLKS, e=GBLK)[
                        :, g // GBLK, g % GBLK, :]
                else:
                    dst = hT[:, j, :].rearrange(
                        "p (a s) -> p s a", s=NT)[:, g, :]
                if XBAR_T:
                    stg = stgp.tile([P, P], BF16, tag="stg", name="stg")
                    nc.sync.dma_start_transpose(stg[:], ht[:, ts(j, P)])
                    nc.gpsimd.tensor_copy(dst, stg[:])
                else:
                    pt = ps_t.tile([P, P], BF16, tag="pt", name="pt")
                    nc.tensor.transpose(pt[:], ht[:, ts(j, P)], ident[:])
                    if (g + j) % 2 == 0:
                        nc.scalar.copy(dst, pt[:])
                    else:
                        nc.vector.tensor_copy(dst, pt[:])

        for b in range(nbatch):
            xg = x_d[b].rearrange("(a s) c -> a s c", s=NT)
            yg = y_d[b].rearrange("(a s) c -> a s c", s=NT)

            # ---- load x (grouped token tiles) in halves ----
            xh = []
            for hf in range(2):
                xt = xres.tile([P, NTH, C], F32, tag="x", name=f"xh{hf}")
                nc.gpsimd.dma_start(xt[:], xg[:, ts(hf, NTH), :])
                xh.append(xt)

            def x_ap(g):
                return xh[g // NTH][:, g % NTH, :]

            # ---- phase A: LN1 + h1T (DMA transpose) + G accumulation ----
            h1T = hTp.tile([P, CK, N], BF16, tag="hT", name="h1T")
            psG = [ps_big.tile([P, 512], F32, tag="big", name=f"psG{j}")
                   for j in range(CK)]
            for g in range(NT):
                ht = htp.tile([P, C], BF16, tag="ht", name="ht")
                layernorm(x_ap(g), ht[:])
                ln_transpose(ht, h1T, g, False)
                for j in range(CK):
                    nc.tensor.matmul(psG[j][:, :C], ht[:, ts(j, P)], ht[:, :C],
                                     start=(g == 0), stop=(g == NT - 1))

            # ---- G -> SBUF (f32), T = G @ [Wq|Wk] in f32r ----
            Gsb = gsb.tile([P, CK, C], GRAM_DT, tag="G", name="Gsb")
            for j in range(CK):
                if j % 2 == 0:
                    nc.vector.tensor_copy(Gsb[:, j, :], psG[j][:, :C])
                else:
                    nc.scalar.copy(Gsb[:, j, :], psG[j][:, :C])
            Tsb = tsb.tile([P, CK, 2 * C], GRAM_DT, tag="T", name="Tsb")
            for m in range(CK):
                for hf in range(2):
                    pT = ps_big.tile([P, 512], F32, tag="big", name="pT")
                    for jj in range(CK):
                        nc.tensor.matmul(
                            pT[:, :C], Gsb[:, jj, ts(m, P)],
                            wqk_sb[:, jj, ts(hf, C)],
                            start=(jj == 0), stop=(jj == CK - 1))
                    dst = Tsb[:, m, ts(hf, C)]
                    if (m * 2 + hf) % 2 == 0:
                        nc.vector.tensor_copy(dst, pT[:, :C])
                    else:
                        nc.scalar.copy(dst, pT[:, :C])

            # ---- per-head V_h = U_h^T T_h: scores (S^T) + norms (diag) ----
            norms2 = tiny.tile([2 * HD, H], F32, tag="norms", name="norms2")
            sTall = tiny.tile([HD, H, HD], F32, tag="sTall", name="sTall")
            for h in range(H):
                psV = ps_v.tile([2 * HD, 2 * HD], F32, tag="V", name="psV")
                for jj in range(CK):
                    nc.tensor.matmul(psV[:], wqk_sb[:, jj, ts(h, 2 * HD)],
                                     Tsb[:, jj, ts(h, 2 * HD)],
                                     start=(jj == 0), stop=(jj == CK - 1))
                nc.scalar.copy(sTall[:, h, :], psV[0:HD, HD:2 * HD])
                junk = tiny.tile([2 * HD, 2 * HD], F32, tag="junk", name="junk")
                nc.vector.tensor_tensor(junk[:], psV[:],
                                        ident[0:2 * HD, 0:2 * HD], ALU.mult)
                junk2 = tiny.tile([2 * HD, 2 * HD], F32, tag="junk2",
                                  name="junk2")
                nc.scalar.activation(junk2[:], junk[:], AF.Identity,
                                     accum_out=norms2[:, h:h + 1])

            # ---- l2norm factors: rs = rsqrt(max(n2, eps)), k-side * SCALE ----
            rsall = tiny.tile([2 * HD, H], F32, tag="rsall", name="rsall")
            nc.vector.tensor_scalar_max(rsall[:], norms2[:], 1e-24)
            nc.scalar.activation(rsall[:], rsall[:], AF.Sqrt)
            nc.vector.reciprocal(rsall[:], rsall[:])
            # SCALE folded into the q-side factors (partition base 0; a
            # base-48 compute slice would be rejected by the verifier)
            nc.vector.tensor_scalar_mul(rsall[0:HD, :], rsall[0:HD, :], SCALE)
            # broadcast k-side factors along partitions via DRAM roundtrip
            # DRAM layout h-major (h*48 + d) so the broadcast reload is a
            # plain 2D stride-0-partition AP.
            s_dram = dram.tile([HD * H], F32, tag="s_dram", name="s_dram")
            _sd = s_dram[:]
            nc.gpsimd.dma_start(
                bass.AP(tensor=_sd.tensor, offset=_sd.offset,
                        ap=[[1, HD], [HD, H]]), rsall[HD:2 * HD, :])
            nc.gpsimd.dma_start(
                skb[:], bass.AP(tensor=_sd.tensor, offset=_sd.offset,
                                ap=[[0, HD], [1, H * HD]]))

            # ---- softmax + M^T columns (mftT), transposed via XBAR ----
            mftT = mftp.tile([P, CK, C], BF16, tag="mftT", name="mftT")
            for h in range(H):
                t1 = tiny.tile([HD, HD], F32, tag="t1", name="t1")
                nc.vector.scalar_tensor_tensor(
                    t1[:], sTall[:, h, :], rsall[0:HD, h:h + 1], skb[:, h, :],
                    op0=ALU.mult, op1=ALU.mult)
                e1T = tiny.tile([HD, HD], BF16, tag="e1T", name="e1T")
                nc.scalar.activation(e1T[:], t1[:], AF.Exp)
                psm = ps_m.tile([HD, C + 1], F32, tag="pm", name="psm")
                nc.tensor.matmul(psm[:], e1T[:], wvt_sb[:, h, :],
                                 start=True, stop=True)
                rsd = tiny.tile([HD, 1], F32, tag="rsd", name="rsd")
                nc.vector.reciprocal(rsd[:], psm[:, C:C + 1])
                mp = tiny.tile([HD, C], BF16, tag="mp", name="mp")
                nc.vector.tensor_scalar(mp[:], psm[:, 0:C],
                                        scalar1=rsd[:], scalar2=None,
                                        op0=ALU.mult)
                for kc in range(CK):
                    dst = mftT[:, kc, :].rearrange(
                        "p (d e) -> p e d", e=H)[:, h, :]
                    if XBAR_T:
                        stg = stgp.tile([P, P], BF16, tag="stg", name="stgm")
                        nc.sync.dma_start_transpose(stg[:, 0:HD],
                                                    mp[:, ts(kc, P)])
                        nc.gpsimd.tensor_copy(dst, stg[:, 0:HD])
                    else:
                        pt = ps_t.tile([P, P], BF16, tag="pt", name="ptm")
                        nc.tensor.transpose(pt[:, 0:HD], mp[:, ts(kc, P)],
                                            ident[0:HD, 0:HD])
                        if (h + kc) % 2 == 0:
                            nc.scalar.copy(dst, pt[:, 0:HD])
                        else:
                            nc.vector.tensor_copy(dst, pt[:, 0:HD])

            # ---- ZT[n, r] = sum_c h^T[c, n] M^T[c, r] (token-major) ----
            ZT = bigp.tile([P, NT, C], BF16, tag="big", name="ZT")
            for mu in range(NT):
                pz = ps_big.tile([P, 512], F32, tag="big", name="pz")
                for kc in range(CK):
                    nc.tensor.matmul(pz[:, :C], h1T[:, kc, ts(mu, P)],
                                     mftT[:, kc, :],
                                     start=(kc == 0), stop=(kc == CK - 1))
                if mu % 2 == 0:
                    nc.vector.tensor_copy(ZT[:, mu, :], pz[:, :C])
                else:
                    nc.scalar.copy(ZT[:, mu, :], pz[:, :C])

            # ---- proj via stride-3 ZT slices + residual + LN2 + h2T ----
            h2T = hTp.tile([P, CK, N], BF16, tag="hT", name="h2T")
            for g in range(NT):
                pp = ps_big.tile([P, 512], F32, tag="big", name="pp")
                for kj in range(CK):
                    q3 = 3 * g + kj
                    rho, mu = q3 // NT, q3 % NT
                    lhsT = ZT[:, mu, :].rearrange(
                        "p (a t) -> p t a", t=CK)[:, rho, :]
                    nc.tensor.matmul(pp[:, :C], lhsT, wpr_sb[:, kj, :],
                                     start=(kj == 0), stop=(kj == CK - 1))
                nc.vector.tensor_add(x_ap(g), pp[:, :C], x_ap(g))
                if apply_pjb:
                    nc.vector.tensor_add(x_ap(g), x_ap(g), pjb_sb[:])
                ht2 = htp.tile([P, C], BF16, tag="ht", name="ht2")
                layernorm(x_ap(g), ht2[:])
                ln_transpose(ht2, h2T, g, True)

            # ---- MLP in 8-group blocks: fc1 -> gelu -> fc2 (token-major) --
            for blk in range(NBLKS):
                g0 = blk * GBLK
                g4 = bigp.tile([P, HK, BTOK], f2dt, tag="big", name="g4")
                for m in range(HK):
                    pf = [ps_big.tile([P, 512], F32, tag="big", name=f"pf{i}")
                          for i in range(2)]
                    for kj in range(CK):
                        for i in range(2):
                            rhs = h2T[:, kj, blk * BTOK + i * 512:
                                      blk * BTOK + (i + 1) * 512]
                            nc.tensor.matmul(pf[i][:], wf1_sb[:, kj, ts(m, P)],
                                             rhs, start=(kj == 0),
                                             stop=(kj == CK - 1))
                    for i in range(2):
                        nc.scalar.activation(g4[:, m, ts(i, 512)], pf[i][:],
                                             AF.Gelu, bias=f1b_sb[:, m:m + 1],
                                             scale=1.0)
                g4s = g4.rearrange("p k (a e) -> p k e a", e=GBLK)
                for gg in range(GBLK):
                    g = g0 + gg
                    pf2 = ps_big.tile([P, 512], F32, tag="big", name="pf2")
                    if FP8_MLP:
                        g4e = g4.rearrange("p k (a e) -> p k e a", e=GBLK)
                        for u in range(HK // 2):
                            nc.tensor.matmul(
                                pf2[:, :C], g4e[:, 2 * u:2 * u + 2, gg, :],
                                wf2_sb[:, 2 * u:2 * u + 2, :],
                                start=(u == 0), stop=(u == HK // 2 - 1),
                                perf_mode=DR)
                    else:
                        for kj in range(HK):
                            nc.tensor.matmul(pf2[:, :C], g4s[:, kj, gg, :],
                                             wf2_sb[:, kj, :],
                                             start=(kj == 0),
                                             stop=(kj == HK - 1))
                    yt = outp.tile([P, C], F32, tag="yt", name="yt")
                    if FP8_MLP:
                        # descale the x64 fp8 weight prescale, add bias
                        nc.vector.scalar_tensor_tensor(
                            yt[:], pf2[:, :C], 1.0 / 64.0, f2b_sb[:],
                            op0=ALU.mult, op1=ALU.add)
                    else:
                        nc.vector.tensor_tensor(yt[:], pf2[:, :C], f2b_sb[:],
                                                ALU.add)
                    nc.vector.tensor_add(yt[:], yt[:], x_ap(g))
                    nc.sync.dma_start(yg[:, g, :], yt[:])

    nc.compile()
    return nc


def kernel_gram_np_dtype():
    return ml_dtypes.bfloat16 if GRAM_DT == mybir.dt.bfloat16 else None


def _prep_inputs(x, qkv_w, qkv_b, proj_w, proj_b, n1_g, n1_b, n2_g, n2_b,
                 fc1_w, fc1_b, fc2_w, fc2_b):
    """Host-side folding of LN affine params into the adjacent matmuls."""
    bf = ml_dtypes.bfloat16
    x = np.ascontiguousarray(np.asarray(x, np.float32))
    qkv_w = np.asarray(qkv_w, np.float32)
    qkv_b = np.asarray(qkv_b, np.float32)
    n1_g = np.asarray(n1_g, np.float32)
    n1_b = np.asarray(n1_b, np.float32)
    fc1_w = np.asarray(fc1_w, np.float32)
    wqk = n1_g[:, None] * qkv_w[:, :2 * C]
    # permute columns to per-head [q48 | k48] blocks (h*96 + {d, 48+d}) so
    # the V_h matmuls read contiguous slices
    perm = np.concatenate(
        [np.concatenate([h * HD + np.arange(HD), C + h * HD + np.arange(HD)])
         for h in range(H)])
    wqk = np.ascontiguousarray(wqk[:, perm])
    if kernel_gram_np_dtype() is not None:
        wqk = wqk.astype(kernel_gram_np_dtype())
    qkb = qkv_b[:2 * C] + n1_b @ qkv_w[:, :2 * C]
    if np.any(qkb != 0):
        raise NotImplementedError("nonzero q/k-bias not supported")
    wv = n1_g[:, None] * qkv_w[:, 2 * C:]
    vb = qkv_b[2 * C:] + n1_b @ qkv_w[:, 2 * C:]
    if np.any(vb != 0):
        raise NotImplementedError("nonzero v-bias not supported")
    wvt = np.concatenate(
        [np.ascontiguousarray(wv.T), np.ones((C, 1), np.float32)],
        axis=1).astype(bf)
    wf1 = (np.asarray(n2_g, np.float32)[:, None] * fc1_w).astype(bf)
    f1b = np.asarray(fc1_b, np.float32) + np.asarray(n2_b, np.float32) @ fc1_w
    pjb = np.asarray(proj_b, np.float32)
    apply_pjb = bool(np.any(pjb != 0))
    wf2 = np.asarray(fc2_w, np.float32)
    if FP8_MLP:
        wf2 = (wf2 * 64.0).astype(ml_dtypes.float8_e4m3)
    else:
        wf2 = wf2.astype(bf)
    common = {
        "wqk": wqk, "wvt": wvt,
        "wpr": np.asarray(proj_w, np.float32).astype(bf), "pjb": pjb,
        "wf1": wf1, "f1b": f1b.astype(np.float32),
        "wf2": wf2, "f2b": np.asarray(fc2_b, np.float32),
    }
    in_maps = []
    for c in range(NCORES):
        m = dict(common)
        m["x"] = x[c * BL:(c + 1) * BL]
        in_maps.append(m)
    return in_maps, apply_pjb


_CACHE = {}


def run(inputs: dict, trace: bool = False):
    in_maps, apply_pjb = _prep_inputs(**inputs)
    key = (apply_pjb,)
    if key not in _CACHE:
        _CACHE[key] = build_program(apply_pjb)
    nc = _CACHE[key]
    res = run_bass_kernel_spmd(nc, in_maps, core_ids=list(range(NCORES)),
                               trace=trace)
    y = np.concatenate([res.results[c]["y"] for c in range(NCORES)], axis=0)
    return y.astype(np.float32), res


def kernel(**inputs) -> np.ndarray:
    y, _ = run(inputs, trace=False)
    return y
